# revision 5
# baseline (speedup 1.0000x reference)
"""GRU-cell (task-specific, 3-source gates) Trainium2 kernel.

Math (per reference):
    gx = x @ Wx + bx; gh = h_prev @ Wh + bh; gs = h_shared_new @ Ws + bs
    z = sigmoid(gx_z + gh_z + gs_z)
    r = sigmoid(gx_r + gh_r + gs_r)
    n = tanh(gx_n + r * gh_n + gs_n)
    h_new = (1 - z) * n + z * h_prev

Strategy: pure data-parallel across 8 NeuronCores (batch 16384 -> 2048/core),
weights replicated. On each core, for each 128-row batch tile and each group
of 256 H-columns, the z and r pre-activations of all three GEMMs accumulate
into a single PSUM bank (PE does the adds); the n-part keeps (x@Wx + hs@Ws)
and (h@Wh) in separate banks since r multiplies only the latter. Gate
elementwise runs on DVE/ACT while PE streams the next tile's matmuls.

Matmuls run as float32r (fp32 transpose-mode) at 1 cycle/row for N>=256 —
full fp32 precision at bf16-rate streaming.

Activations are fed feature-major (transposed on host during sharding) so
128x128 fp32 blocks can load straight into the stationary operand.
"""

import numpy as np

import concourse.bacc as bacc
import concourse.bass as bass
import concourse.mybir as mybir
import concourse.tile as tile
from concourse.bass_utils import run_bass_kernel_spmd

B = 16384
D = 1024
H = 1024
H3 = 3072
NCORES = 8
BC = B // NCORES          # 2048 batch rows per core
P = 128                   # partitions
BT = BC // P              # 16 batch tiles per core
KC = D // P               # 8 contraction chunks
NG = 4                    # H-column groups
GW = H // NG              # 256 H-columns per group
GW3 = 3 * GW              # 768 gate columns per group (z|r|n slices)

F32 = mybir.dt.float32
F32R = mybir.dt.float32r
AF = mybir.ActivationFunctionType


def _build_bass(trace_label: str = "grucell"):
    nc = bacc.Bacc("TRN2", target_bir_lowering=False)

    # Per-core DRAM I/O (feature-major activations, group-packed weights).
    xt = nc.dram_tensor("xt", [D, BC], F32R, kind="ExternalInput")
    ht = nc.dram_tensor("ht", [D, BC], F32R, kind="ExternalInput")
    st = nc.dram_tensor("st", [D, BC], F32R, kind="ExternalInput")
    hp = nc.dram_tensor("hp", [BC, H], F32, kind="ExternalInput")
    wxp = nc.dram_tensor("wxp", [NG, D, GW3], F32R, kind="ExternalInput")
    whp = nc.dram_tensor("whp", [NG, D, GW3], F32R, kind="ExternalInput")
    wsp = nc.dram_tensor("wsp", [NG, D, GW3], F32R, kind="ExternalInput")
    # bzr: bias (bx+bh+bs) for [z|r] slices; bn: [bx+bs | bh] for n slice.
    bzr = nc.dram_tensor("bzr", [NG, P, 2 * GW], F32, kind="ExternalInput")
    bn = nc.dram_tensor("bn", [NG, P, 2 * GW], F32, kind="ExternalInput")
    out = nc.dram_tensor("out", [BC, H], F32, kind="ExternalOutput")

    with tile.TileContext(nc) as tc:
        with (
            tc.tile_pool(name="w", bufs=2) as wpool,
            tc.tile_pool(name="bias", bufs=2) as bpool,
            tc.tile_pool(name="acts", bufs=2) as apool,
            tc.tile_pool(name="hprev", bufs=2) as hpool,
            tc.tile_pool(name="gates", bufs=2) as gpool,
            tc.tile_pool(name="outs", bufs=2) as opool,
            tc.tile_pool(name="ps", bufs=3, space="PSUM") as pspool,
        ):
            for g in range(NG):
                wx_t = wpool.tile([P, KC, GW3], F32R, tag="wx")
                wh_t = wpool.tile([P, KC, GW3], F32R, tag="wh")
                ws_t = wpool.tile([P, KC, GW3], F32R, tag="ws")
                nc.sync.dma_start(
                    wx_t[:], wxp[g].rearrange("(kc p) c -> p kc c", p=P))
                nc.sync.dma_start(
                    wh_t[:], whp[g].rearrange("(kc p) c -> p kc c", p=P))
                nc.sync.dma_start(
                    ws_t[:], wsp[g].rearrange("(kc p) c -> p kc c", p=P))
                bzr_t = bpool.tile([P, 2 * GW], F32, tag="bzr")
                bn_t = bpool.tile([P, 2 * GW], F32, tag="bn")
                nc.sync.dma_start(bzr_t[:], bzr[g])
                nc.sync.dma_start(bn_t[:], bn[g])

                for bt in range(BT):
                    bsl = bass.ts(bt, P)
                    xt_t = apool.tile([P, KC, P], F32R, tag="xt")
                    ht_t = apool.tile([P, KC, P], F32R, tag="ht")
                    st_t = apool.tile([P, KC, P], F32R, tag="st")
                    nc.sync.dma_start(
                        xt_t[:], xt[:, bsl].rearrange("(kc p) b -> p kc b", p=P))
                    nc.sync.dma_start(
                        ht_t[:], ht[:, bsl].rearrange("(kc p) b -> p kc b", p=P))
                    nc.sync.dma_start(
                        st_t[:], st[:, bsl].rearrange("(kc p) b -> p kc b", p=P))
                    hp_t = hpool.tile([P, GW], F32, tag="hp")
                    nc.sync.dma_start(hp_t[:], hp[bsl, bass.ts(g, GW)])

                    # PSUM: AB = [z|r] pre-acts (all 3 sources summed by PE);
                    # CD = [x@Wx+hs@Ws | h@Wh] for the n slice.
                    ab = pspool.tile([P, 2 * GW], F32, tag="ab")
                    cd = pspool.tile([P, 2 * GW], F32, tag="cd")

                    # z|r halves: 3 sources x 8 kc each.
                    for half, c0 in ((0, 0), (1, GW)):
                        dst = ab[:, bass.ts(half, GW)]
                        srcs = ((xt_t, wx_t), (st_t, ws_t), (ht_t, wh_t))
                        n_mm = len(srcs) * KC
                        i = 0
                        for act_t, w_t in srcs:
                            for kc in range(KC):
                                nc.tensor.matmul(
                                    dst,
                                    act_t[:, kc, :],
                                    w_t[:, kc, c0:c0 + GW],
                                    start=(i == 0),
                                    stop=(i == n_mm - 1),
                                )
                                i += 1
                    # n slice: C = x@Wx_n + hs@Ws_n ; D = h@Wh_n.
                    i = 0
                    for act_t, w_t in ((xt_t, wx_t), (st_t, ws_t)):
                        for kc in range(KC):
                            nc.tensor.matmul(
                                cd[:, 0:GW],
                                act_t[:, kc, :],
                                w_t[:, kc, 2 * GW:3 * GW],
                                start=(i == 0),
                                stop=(i == 2 * KC - 1),
                            )
                            i += 1
                    for kc in range(KC):
                        nc.tensor.matmul(
                            cd[:, GW:2 * GW],
                            ht_t[:, kc, :],
                            wh_t[:, kc, 2 * GW:3 * GW],
                            start=(kc == 0),
                            stop=(kc == KC - 1),
                        )

                    # Gate elementwise. zr = sigmoid(AB + b); n needs
                    # tanh(Sn + r*Gn) with biases added first.
                    zr_s = gpool.tile([P, 2 * GW], F32, tag="zr_s")
                    nc.vector.tensor_add(zr_s[:], ab[:], bzr_t[:])
                    zr = gpool.tile([P, 2 * GW], F32, tag="zr")
                    nc.scalar.activation(zr[:], zr_s[:], AF.Sigmoid)
                    cd_s = gpool.tile([P, 2 * GW], F32, tag="cd_s")
                    nc.vector.tensor_add(cd_s[:], cd[:], bn_t[:])
                    m_t = gpool.tile([P, GW], F32, tag="m")
                    nc.vector.tensor_mul(
                        m_t[:], zr[:, GW:2 * GW], cd_s[:, GW:2 * GW])
                    u_t = gpool.tile([P, GW], F32, tag="u")
                    nc.vector.tensor_add(u_t[:], m_t[:], cd_s[:, 0:GW])
                    nm_t = gpool.tile([P, GW], F32, tag="nm")
                    nc.scalar.activation(nm_t[:], u_t[:], AF.Tanh)
                    # h = nm + z*(h_prev - nm)
                    d_t = gpool.tile([P, GW], F32, tag="d")
                    nc.vector.tensor_sub(d_t[:], hp_t[:], nm_t[:])
                    e_t = gpool.tile([P, GW], F32, tag="e")
                    nc.vector.tensor_mul(e_t[:], zr[:, 0:GW], d_t[:])
                    ho_t = opool.tile([P, GW], F32, tag="ho")
                    nc.vector.tensor_add(ho_t[:], nm_t[:], e_t[:])
                    nc.sync.dma_start(out[bsl, bass.ts(g, GW)], ho_t[:])
    nc.compile()
    return nc


_CACHED_NC = None


def _get_nc():
    global _CACHED_NC
    if _CACHED_NC is None:
        _CACHED_NC = _build_bass()
    return _CACHED_NC


def _prep_inputs(x, h_prev, h_shared_new, Wx, Wh, Ws, bx, bh, bs):
    """Host-side shard + layout prep. Returns per-core in_maps."""
    f = np.float32
    xt = np.ascontiguousarray(x.T, dtype=f)
    ht = np.ascontiguousarray(h_prev.T, dtype=f)
    st = np.ascontiguousarray(h_shared_new.T, dtype=f)

    def pack_w(W):
        Wg = np.empty((NG, D, GW3), dtype=f)
        for g in range(NG):
            sl = slice(g * GW, (g + 1) * GW)
            Wg[g] = np.concatenate(
                [W[:, 0:H][:, sl], W[:, H:2 * H][:, sl], W[:, 2 * H:3 * H][:, sl]],
                axis=1)
        return Wg

    wxp, whp, wsp = pack_w(np.asarray(Wx, f)), pack_w(np.asarray(Wh, f)), \
        pack_w(np.asarray(Ws, f))
    b_sum = (np.asarray(bx, f) + np.asarray(bh, f) + np.asarray(bs, f))
    b_xs = (np.asarray(bx, f) + np.asarray(bs, f))
    bzr = np.empty((NG, P, 2 * GW), dtype=f)
    bn = np.empty((NG, P, 2 * GW), dtype=f)
    for g in range(NG):
        sl = slice(g * GW, (g + 1) * GW)
        bzr[g] = np.concatenate([b_sum[0:H][sl], b_sum[H:2 * H][sl]])[None, :]
        bn[g] = np.concatenate([b_xs[2 * H:][sl], np.asarray(bh, f)[2 * H:][sl]])[None, :]

    in_maps = []
    for c in range(NCORES):
        rows = slice(c * BC, (c + 1) * BC)
        in_maps.append({
            "xt": np.ascontiguousarray(xt[:, rows]),
            "ht": np.ascontiguousarray(ht[:, rows]),
            "st": np.ascontiguousarray(st[:, rows]),
            "hp": np.ascontiguousarray(np.asarray(h_prev, f)[rows]),
            "wxp": wxp, "whp": whp, "wsp": wsp,
            "bzr": bzr, "bn": bn,
        })
    return in_maps


def kernel(x, h_prev, h_shared_new, Wx, Wh, Ws, bx, bh, bs, _trace=False):
    nc = _get_nc()
    in_maps = _prep_inputs(x, h_prev, h_shared_new, Wx, Wh, Ws, bx, bh, bs)
    res = run_bass_kernel_spmd(nc, in_maps, list(range(NCORES)), trace=_trace)
    out = np.concatenate([r["out"] for r in res.results], axis=0)
    if _trace:
        kernel.last_results = res
    return out.astype(np.float32)
